# revision 2
# baseline (speedup 1.0000x reference)
"""Trainium2 Bass kernel for the additive-attention module — fp8 variant.

Differences from the bf16 baseline (kernel.py):
  - The dominant matmul (att_enc = enc @ W_enc) runs in fp8e4 DoubleRow mode:
    256-deep contraction tiles, ~2x PE throughput.
  - W_enc is pre-scaled (x512), pair-interleaved, and cast to fp8 on the HOST
    (weights are small); shipped as uint8 and bitcast on chip.
    W8[p, dj, i, m] = W_enc[256 dj + 2p + i, m] * 512.
  - enc is cast bf16 -> fp8 on chip (DVE/Pool alternating), and the SBUF->SBUF
    xbar transposes move the fp8 data as u16 pairs: after transposing
    natf8.bitcast(bf16), fp8 element (p, 2r+i) of a 256-wide e-block equals
    enc[r, 256 dj + 2p + i] — exactly matching W8's pair layout.
    Transpose traffic halves vs bf16 (12.8 MB vs 25.7 MB per core).
  - bias+relu absorbs the 1/512 weight scale via the activation scale.
  - step4 (W_fin reduction), softmax, and step6 (weighted sum over bf16 nat)
    are unchanged from the baseline, keeping output precision at bf16 level.
"""

import sys

try:
    import concourse.bass as bass  # noqa: F401
except ImportError:
    sys.path.insert(0, "/opt/trn_rl_repo")

import numpy as np

import concourse.bass as bass
import concourse.mybir as mybir
import concourse.tile as tile
from concourse import bacc
from concourse.bass_utils import run_bass_kernel_spmd
from concourse.masks import make_identity

F32 = mybir.dt.float32
BF16 = mybir.dt.bfloat16
FP8 = mybir.dt.float8e4
U8 = mybir.dt.uint8
AF = mybir.ActivationFunctionType

N_CORES = 8
B = 256
B_LOC = B // N_CORES  # 32
P = 196
E = 2048
A = 512
W = 512
ROWS = B_LOC * P  # 6272
NCHUNK = (ROWS + 127) // 128  # 49
DJ = E // 256  # 8 double-row contraction tiles
AJ = A // 128  # 4
WJ = W // 128  # 4
EG = E // 512  # 4
WSCALE = 512.0


def _batch_segments(r0, nrows):
    """Batch segments of global row range [r0, r0+nrows): (batch, local_s0, local_s1)."""
    segs = []
    b0 = r0 // P
    b1 = (r0 + nrows - 1) // P
    for b in range(b0, b1 + 1):
        s0 = max(b * P - r0, 0)
        s1 = min((b + 1) * P - r0, nrows)
        if s1 > s0:
            segs.append((b, s0, s1))
    return segs


def build(debug_attT=False):
    nc = bacc.Bacc()

    enc_x = nc.dram_tensor("encoder_out", [ROWS, E], F32, kind="ExternalInput")
    attT_x = (
        nc.dram_tensor("attT_dbg", [128, NCHUNK], F32, kind="ExternalOutput")
        if debug_attT
        else None
    )
    dec_x = nc.dram_tensor("decoder_out", [B_LOC, W], F32, kind="ExternalInput")
    w8_x = nc.dram_tensor("W8", [128, DJ * 2 * A], U8, kind="ExternalInput")
    benc_x = nc.dram_tensor("b_enc", [1, A], F32, kind="ExternalInput")
    wdec_x = nc.dram_tensor("W_dec", [W, A], F32, kind="ExternalInput")
    bdec_x = nc.dram_tensor("b_dec", [1, A], F32, kind="ExternalInput")
    wfin_x = nc.dram_tensor("W_fin", [A], F32, kind="ExternalInput")
    out_x = nc.dram_tensor("out", [B_LOC, E], F32, kind="ExternalOutput")

    with tile.TileContext(nc) as tc:
        with tc.tile_pool(name="consts", bufs=1) as consts:
            identity = consts.tile([128, 128], F32)
            make_identity(nc, identity[:])
            wfin_sb = consts.tile([128, AJ], BF16)
            nc.gpsimd.dma_start(wfin_sb[:], wfin_x.rearrange("(j p) -> p j", p=128))
            w8_sb = consts.tile([128, DJ * 2 * A], U8)
            nc.sync.dma_start(w8_sb[:], w8_x[:])
            # w8 as [p, dj, i, m] fp8
            w8_4d = w8_sb[:].bitcast(FP8).rearrange(
                "p (dj two m) -> p dj two m", dj=DJ, two=2
            )
            w_dec_sb = consts.tile([128, WJ * A], BF16)
            nc.gpsimd.dma_start(w_dec_sb[:], wdec_x.rearrange("(j p) a -> p j a", p=128))
            ones32 = consts.tile([1, 32], BF16)
            nc.vector.memset(ones32[:], 1.0)
            onescol = consts.tile([128, 1], BF16)
            nc.vector.memset(onescol[:], 1.0)

            dec_sb = consts.tile([B_LOC, W], F32)
            nc.sync.dma_start(dec_sb[:], dec_x[:])
            benc_sb = consts.tile([1, A], F32)
            nc.sync.dma_start(benc_sb[:], benc_x[:])
            bdec_sb = consts.tile([1, A], F32)
            nc.sync.dma_start(bdec_sb[:], bdec_x[:])
            bb_f = consts.tile([1, A], F32)
            nc.vector.tensor_add(bb_f[:], benc_sb[:], bdec_sb[:])
            bb_bf = consts.tile([1, A], BF16)
            nc.vector.tensor_copy(bb_bf[:], bb_f[:])

            decT_bf = consts.tile([128, WJ * B_LOC], BF16)
            biasT_sb = consts.tile([128, AJ * B_LOC], F32)
            attT_sb = consts.tile([128, NCHUNK], BF16)
            out_sb = consts.tile([B_LOC, E], F32)
            recip_z = consts.tile([B_LOC, 1], F32)

            # per-chunk batch-membership masks (see kernel.py for the scheme)
            id4 = consts.tile([128, B_LOC], F32)
            nc.gpsimd.memset(id4[:], 0.0)
            for k in range(4):
                nc.gpsimd.affine_select(
                    id4[:], id4[:], pattern=[[-1, B_LOC]],
                    compare_op=mybir.AluOpType.not_equal, fill=1.0,
                    base=-B_LOC * k, channel_multiplier=1,
                )
            ones_pb = consts.tile([128, B_LOC], mybir.dt.int8)
            nc.vector.memset(ones_pb[:], 1)
            masks_sb = consts.tile([128, NCHUNK * B_LOC], mybir.dt.int8)

            def issue_mask(c):
                m = masks_sb[:, c * B_LOC : (c + 1) * B_LOC]
                nc.gpsimd.affine_select(
                    m, ones_pb[:], pattern=[[-P, B_LOC]],
                    compare_op=mybir.AluOpType.is_ge, fill=0.0,
                    base=128 * c, channel_multiplier=1,
                )
                nc.gpsimd.affine_select(
                    m, m, pattern=[[P, B_LOC]],
                    compare_op=mybir.AluOpType.is_ge, fill=0.0,
                    base=(P - 1) - 128 * c, channel_multiplier=-1,
                )

            # prologue: decT, then biasT = (dec @ W_dec + b_dec + b_enc).T  [a, b]
            with tc.tile_pool(name="pro_ps", bufs=2, space="PSUM") as pro_ps:
                for j in range(WJ):
                    ps_dt = pro_ps.tile([128, B_LOC], F32, name="ps_dt")
                    nc.tensor.transpose(
                        ps_dt[:], dec_sb[0:B_LOC, j * 128 : (j + 1) * 128],
                        identity[0:B_LOC, 0:B_LOC],
                    )
                    nc.vector.tensor_copy(decT_bf[:, j * B_LOC : (j + 1) * B_LOC], ps_dt[:])
                for aj in range(AJ):
                    ps_b = pro_ps.tile([128, B_LOC], F32, name="ps_b")
                    for wj in range(WJ):
                        nc.tensor.matmul(
                            ps_b[:],
                            lhsT=w_dec_sb[:, wj * A + aj * 128 : wj * A + (aj + 1) * 128],
                            rhs=decT_bf[:, wj * B_LOC : (wj + 1) * B_LOC],
                            start=(wj == 0), stop=False,
                        )
                    nc.tensor.matmul(
                        ps_b[:],
                        lhsT=bb_bf[0:1, aj * 128 : (aj + 1) * 128],
                        rhs=ones32[0:1, :],
                        start=False, stop=True,
                    )
                    nc.scalar.copy(biasT_sb[:, aj * B_LOC : (aj + 1) * B_LOC], ps_b[:])

            with (
                tc.tile_pool(name="nat_pool", bufs=6) as nat_pool,
                tc.tile_pool(name="natf8_pool", bufs=6) as natf8_pool,
                tc.tile_pool(name="encT_pool", bufs=3) as encT_pool,
                tc.tile_pool(name="hidT_pool", bufs=4) as hidT_pool,
                tc.tile_pool(name="w6_pool", bufs=6) as w6_pool,
                tc.tile_pool(name="mm_ps", bufs=2, space="PSUM") as mm_ps,
                tc.tile_pool(name="at_ps_pool", bufs=1, space="PSUM") as at_ps_pool,
                tc.tile_pool(name="acc_ps", bufs=1, space="PSUM") as acc_ps,
            ):
                out_ps = [
                    acc_ps.tile([128, 512], F32, name=f"out_ps{eg}") for eg in range(EG)
                ]
                z_ps = acc_ps.tile([128, 1], F32)

                nat = [None] * NCHUNK
                next6 = 0
                sizes = [1, 1, 2] + [4] * ((NCHUNK - 4) // 4)
                sizes += [NCHUNK - sum(sizes)] if sum(sizes) < NCHUNK else []
                assert sum(sizes) == NCHUNK
                starts = [sum(sizes[:i]) for i in range(len(sizes))]
                for g, (cstart, nch) in enumerate(zip(starts, sizes)):
                    gr = nch * 128
                    # encT8: [p, dj, (r two)] fp8 — pair (i=0,1) adjacent per row
                    encT = encT_pool.tile([128, DJ * 2 * 512], FP8, name="encT")
                    for pc in range(0, nch, 2):
                        c0 = cstart + pc
                        npair = min(2, nch - pc)
                        nat_t = nat_pool.tile([128, 2 * E], BF16, name="nat")
                        for i in range(npair):
                            nat[c0 + i] = nat_t[:, i * E : (i + 1) * E]
                        src = enc_x[c0 * 128 : (c0 + npair) * 128, :].rearrange(
                            "(i p) e -> p i e", p=128, i=npair
                        )
                        dst = nat_t.rearrange("p (i e) -> p i e", i=2)[:, 0:npair, :]
                        nc.gpsimd.dma_start(dst, src)
                        for i in range(npair):
                            rc = pc + i
                            # bf16 -> fp8 cast; alternate engines to split load
                            nf8 = natf8_pool.tile([128, E], FP8, name="natf8")
                            eng = nc.vector if (c0 + i) % 2 == 0 else nc.gpsimd
                            eng.tensor_copy(nf8[:], nat[c0 + i])
                            # u16-pair xbar transpose (one HWDGE ring only)
                            encT_3d = encT[:].bitcast(BF16).rearrange(
                                "p (j r) -> p j r", j=DJ
                            )
                            nc.sync.dma_start(
                                encT_3d[:, :, rc * 128 : rc * 128 + 128],
                                nf8[:].bitcast(BF16),
                                transpose=True,
                            )

                    hidT = hidT_pool.tile([128, AJ * 512], BF16, name="hidT")
                    encT_dj = encT[:].rearrange("p (dj rt) -> p dj rt", dj=DJ)
                    for aj in range(AJ):
                        ps_h = mm_ps.tile([128, 512], F32, name="ps_h")
                        for dj in range(DJ):
                            rhs = encT_dj[:, dj, 0 : 2 * gr].rearrange(
                                "p (r two) -> p two r", two=2
                            )
                            nc.tensor.matmul(
                                ps_h[:, 0:gr],
                                lhsT=w8_4d[:, dj, :, aj * 128 : (aj + 1) * 128],
                                rhs=rhs,
                                start=(dj == 0), stop=(dj == DJ - 1),
                                perf_mode=mybir.MatmulPerfMode.DoubleRow,
                            )
                        for b, s0, s1 in _batch_segments(128 * cstart, gr):
                            nc.scalar.activation(
                                hidT[:, aj * 512 + s0 : aj * 512 + s1],
                                ps_h[:, s0:s1],
                                AF.Relu,
                                bias=biasT_sb[:, aj * B_LOC + b : aj * B_LOC + b + 1],
                                scale=1.0 / WSCALE,
                            )

                    for rc in range(nch):
                        c = cstart + rc
                        at_ps = at_ps_pool.tile([128, 1], F32, name="at_ps")
                        for aj in range(AJ):
                            nc.tensor.matmul(
                                at_ps[:],
                                lhsT=hidT[:, aj * 512 + rc * 128 : aj * 512 + rc * 128 + 128],
                                rhs=wfin_sb[:, aj : aj + 1],
                                start=(aj == 0), stop=(aj == AJ - 1),
                            )
                        nc.scalar.activation(attT_sb[:, c : c + 1], at_ps[:], AF.Exp)

                    rows_done = 128 * (cstart + nch)
                    while next6 < NCHUNK:
                        last_b = (128 * next6 + 127) // P
                        if (last_b + 1) * P > rows_done:
                            break
                        c = next6
                        issue_mask(c)
                        w6 = w6_pool.tile([128, B_LOC], BF16, name="w6")
                        nc.vector.memset(w6[:], 0.0)
                        nc.vector.copy_predicated(
                            w6[:],
                            masks_sb[:, c * B_LOC : (c + 1) * B_LOC],
                            attT_sb[:, c : c + 1].broadcast_to([128, B_LOC]),
                        )
                        sj = (c % 4) * B_LOC
                        for eg in range(EG):
                            nc.tensor.matmul(
                                out_ps[eg][sj : sj + B_LOC, :],
                                lhsT=w6[:],
                                rhs=nat[c][:, eg * 512 : (eg + 1) * 512],
                                start=(c < 4), stop=(c >= NCHUNK - 4),
                                tile_position=(0, sj),
                                skip_group_check=True,
                            )
                        nc.tensor.matmul(
                            z_ps[sj : sj + B_LOC, :], lhsT=w6[:], rhs=onescol[:],
                            start=(c < 4), stop=(c >= NCHUNK - 4),
                            tile_position=(0, sj),
                            skip_group_check=True,
                        )
                        next6 += 1

                assert next6 == NCHUNK
                red_sb = consts.tile([128, EG * 512 + 1], F32, name="red_sb")
                for eg in range(EG):
                    nc.scalar.copy(red_sb[:, eg * 512 : (eg + 1) * 512], out_ps[eg][:])
                nc.vector.tensor_copy(red_sb[:, EG * 512 : EG * 512 + 1], z_ps[:])
                zf_ps = mm_ps.tile([B_LOC, 1], F32, name="ps_h")
                nc.tensor.matmul(
                    zf_ps[:], lhsT=id4[:], rhs=red_sb[:, EG * 512 : EG * 512 + 1],
                    start=True, stop=True,
                )
                nc.vector.reciprocal(recip_z[:], zf_ps[:])
                for eg in range(EG):
                    of_ps = mm_ps.tile([B_LOC, 512], F32, name="ps_h")
                    nc.tensor.matmul(
                        of_ps[:], lhsT=id4[:],
                        rhs=red_sb[:, eg * 512 : (eg + 1) * 512],
                        start=True, stop=True,
                    )
                    nc.scalar.activation(
                        out_sb[:, eg * 512 : (eg + 1) * 512],
                        of_ps[:],
                        AF.Copy,
                        scale=recip_z[:],
                    )
                if attT_x is not None:
                    attT_f = consts.tile([128, NCHUNK], F32, name="attT_f")
                    nc.vector.tensor_copy(attT_f[:], attT_sb[:])
                    nc.sync.dma_start(attT_x[:], attT_f[:])
                nc.sync.dma_start(out_x[:], out_sb[:])

    nc.compile()
    return nc


_NC = None


def _get_nc():
    global _NC
    if _NC is None:
        _NC = build()
    return _NC


def _make_w8(wenc):
    """W8[p, dj, i, m] = W_enc[256 dj + 2p + i, m] * WSCALE, fp8e4 as uint8."""
    fp8np = mybir.dt.np(FP8)
    w = (np.asarray(wenc, dtype=np.float32) * WSCALE).reshape(DJ, 128, 2, A)
    w8 = np.ascontiguousarray(w.transpose(1, 0, 2, 3)).astype(fp8np)
    return w8.reshape(128, DJ * 2 * A).view(np.uint8)


def _in_maps(inputs):
    enc = np.ascontiguousarray(np.asarray(inputs["encoder_out"], dtype=np.float32))
    dec = np.ascontiguousarray(np.asarray(inputs["decoder_out"], dtype=np.float32))
    w8 = _make_w8(inputs["W_enc"])
    benc = np.asarray(inputs["b_enc"], dtype=np.float32).reshape(1, A)
    wdec = np.ascontiguousarray(np.asarray(inputs["W_dec"], dtype=np.float32))
    bdec = np.asarray(inputs["b_dec"], dtype=np.float32).reshape(1, A)
    wfin = np.ascontiguousarray(np.asarray(inputs["W_fin"], dtype=np.float32))

    maps = []
    for i in range(N_CORES):
        maps.append(
            {
                "encoder_out": np.ascontiguousarray(
                    enc[i * B_LOC : (i + 1) * B_LOC].reshape(ROWS, E)
                ),
                "decoder_out": np.ascontiguousarray(dec[i * B_LOC : (i + 1) * B_LOC]),
                "W8": w8,
                "b_enc": benc,
                "W_dec": wdec,
                "b_dec": bdec,
                "W_fin": wfin,
            }
        )
    return maps


def run(inputs, trace=False):
    """Run the kernel; returns (out [256, 2048] f32, exec_time_ns or None)."""
    nc = _get_nc()
    res = run_bass_kernel_spmd(
        nc, _in_maps(inputs), core_ids=list(range(N_CORES)), trace=trace
    )
    out = np.concatenate([res.results[i]["out"] for i in range(N_CORES)], axis=0)
    return out.astype(np.float32), res.exec_time_ns


def kernel(**inputs):
    out, _ = run(inputs, trace=False)
    return out


# revision 3
# speedup vs baseline: 1.0406x; 1.0406x over previous
"""Trainium2 Bass kernel for the additive-attention module (fp8 DoubleRow).

Data-parallel over batch across 8 cores (32 batches each); weights replicated.
Key design points vs a bf16 implementation:
  - The dominant matmul (att_enc = enc @ W_enc) runs in fp8e4 DoubleRow mode:
    256-deep contraction tiles, ~2x PE throughput.
  - W_enc is pre-scaled (x512), pair-interleaved, and cast to fp8 on the HOST
    (weights are small); shipped as uint8 and bitcast on chip.
    W8[p, dj, i, m] = W_enc[256 dj + 2p + i, m] * 512.
  - enc is cast bf16 -> fp8 on chip (DVE/Pool alternating), and the SBUF->SBUF
    xbar transposes move the fp8 data as u16 pairs: after transposing
    natf8.bitcast(bf16), fp8 element (p, 2r+i) of a 256-wide e-block equals
    enc[r, 256 dj + 2p + i] — exactly matching W8's pair layout.
    Transpose traffic halves vs bf16 (12.8 MB vs 25.7 MB per core).
  - bias+relu absorbs the 1/512 weight scale via the activation scale.
  - step4 (W_fin reduction), softmax, and step6 (weighted sum over bf16 nat)
    are unchanged from the baseline, keeping output precision at bf16 level.
"""

import sys

try:
    import concourse.bass as bass  # noqa: F401
except ImportError:
    sys.path.insert(0, "/opt/trn_rl_repo")

import numpy as np

import concourse.bass as bass
import concourse.mybir as mybir
import concourse.tile as tile
from concourse import bacc
from concourse.bass_utils import run_bass_kernel_spmd
from concourse.masks import make_identity

F32 = mybir.dt.float32
BF16 = mybir.dt.bfloat16
FP8 = mybir.dt.float8e4
U8 = mybir.dt.uint8
AF = mybir.ActivationFunctionType

N_CORES = 8
B = 256
B_LOC = B // N_CORES  # 32
P = 196
E = 2048
A = 512
W = 512
ROWS = B_LOC * P  # 6272
NCHUNK = (ROWS + 127) // 128  # 49
DJ = E // 256  # 8 double-row contraction tiles
AJ = A // 128  # 4
WJ = W // 128  # 4
EG = E // 512  # 4
WSCALE = 512.0


def _batch_segments(r0, nrows):
    """Batch segments of global row range [r0, r0+nrows): (batch, local_s0, local_s1)."""
    segs = []
    b0 = r0 // P
    b1 = (r0 + nrows - 1) // P
    for b in range(b0, b1 + 1):
        s0 = max(b * P - r0, 0)
        s1 = min((b + 1) * P - r0, nrows)
        if s1 > s0:
            segs.append((b, s0, s1))
    return segs


def build(debug_attT=False):
    nc = bacc.Bacc()

    enc_x = nc.dram_tensor("encoder_out", [ROWS, E], F32, kind="ExternalInput")
    attT_x = (
        nc.dram_tensor("attT_dbg", [128, NCHUNK], F32, kind="ExternalOutput")
        if debug_attT
        else None
    )
    dec_x = nc.dram_tensor("decoder_out", [B_LOC, W], F32, kind="ExternalInput")
    w8_x = nc.dram_tensor("W8", [128, DJ * 2 * A], U8, kind="ExternalInput")
    benc_x = nc.dram_tensor("b_enc", [1, A], F32, kind="ExternalInput")
    wdec_x = nc.dram_tensor("W_dec", [W, A], F32, kind="ExternalInput")
    bdec_x = nc.dram_tensor("b_dec", [1, A], F32, kind="ExternalInput")
    wfin_x = nc.dram_tensor("W_fin", [A], F32, kind="ExternalInput")
    out_x = nc.dram_tensor("out", [B_LOC, E], F32, kind="ExternalOutput")

    with tile.TileContext(nc) as tc:
        with tc.tile_pool(name="consts", bufs=1) as consts:
            identity = consts.tile([128, 128], F32)
            make_identity(nc, identity[:])
            wfin_sb = consts.tile([128, AJ], BF16)
            nc.gpsimd.dma_start(wfin_sb[:], wfin_x.rearrange("(j p) -> p j", p=128))
            w8_sb = consts.tile([128, DJ * 2 * A], U8)
            nc.sync.dma_start(w8_sb[:], w8_x[:])
            # w8 as [p, dj, i, m] fp8
            w8_4d = w8_sb[:].bitcast(FP8).rearrange(
                "p (dj two m) -> p dj two m", dj=DJ, two=2
            )
            w_dec_sb = consts.tile([128, WJ * A], BF16)
            nc.gpsimd.dma_start(w_dec_sb[:], wdec_x.rearrange("(j p) a -> p j a", p=128))
            ones32 = consts.tile([1, 32], BF16)
            nc.vector.memset(ones32[:], 1.0)
            onescol = consts.tile([128, 1], BF16)
            nc.vector.memset(onescol[:], 1.0)

            dec_sb = consts.tile([B_LOC, W], F32)
            nc.sync.dma_start(dec_sb[:], dec_x[:])
            benc_sb = consts.tile([1, A], F32)
            nc.sync.dma_start(benc_sb[:], benc_x[:])
            bdec_sb = consts.tile([1, A], F32)
            nc.sync.dma_start(bdec_sb[:], bdec_x[:])
            bb_f = consts.tile([1, A], F32)
            nc.vector.tensor_add(bb_f[:], benc_sb[:], bdec_sb[:])
            bb_bf = consts.tile([1, A], BF16)
            nc.vector.tensor_copy(bb_bf[:], bb_f[:])

            decT_bf = consts.tile([128, WJ * B_LOC], BF16)
            biasT_sb = consts.tile([128, AJ * B_LOC], F32)
            attT_sb = consts.tile([128, NCHUNK], BF16)
            out_sb = consts.tile([B_LOC, E], F32)
            recip_z = consts.tile([B_LOC, 1], F32)

            # per-chunk batch-membership masks (see kernel.py for the scheme)
            id4 = consts.tile([128, B_LOC], F32)
            nc.gpsimd.memset(id4[:], 0.0)
            for k in range(4):
                nc.gpsimd.affine_select(
                    id4[:], id4[:], pattern=[[-1, B_LOC]],
                    compare_op=mybir.AluOpType.not_equal, fill=1.0,
                    base=-B_LOC * k, channel_multiplier=1,
                )
            ones_pb = consts.tile([128, B_LOC], mybir.dt.int8)
            nc.vector.memset(ones_pb[:], 1)
            masks_sb = consts.tile([128, NCHUNK * B_LOC], mybir.dt.int8)

            def issue_mask(c):
                m = masks_sb[:, c * B_LOC : (c + 1) * B_LOC]
                nc.gpsimd.affine_select(
                    m, ones_pb[:], pattern=[[-P, B_LOC]],
                    compare_op=mybir.AluOpType.is_ge, fill=0.0,
                    base=128 * c, channel_multiplier=1,
                )
                nc.gpsimd.affine_select(
                    m, m, pattern=[[P, B_LOC]],
                    compare_op=mybir.AluOpType.is_ge, fill=0.0,
                    base=(P - 1) - 128 * c, channel_multiplier=-1,
                )

            # prologue: decT, then biasT = (dec @ W_dec + b_dec + b_enc).T  [a, b]
            with tc.tile_pool(name="pro_ps", bufs=2, space="PSUM") as pro_ps:
                for j in range(WJ):
                    ps_dt = pro_ps.tile([128, B_LOC], F32, name="ps_dt")
                    nc.tensor.transpose(
                        ps_dt[:], dec_sb[0:B_LOC, j * 128 : (j + 1) * 128],
                        identity[0:B_LOC, 0:B_LOC],
                    )
                    nc.vector.tensor_copy(decT_bf[:, j * B_LOC : (j + 1) * B_LOC], ps_dt[:])
                for aj in range(AJ):
                    ps_b = pro_ps.tile([128, B_LOC], F32, name="ps_b")
                    for wj in range(WJ):
                        nc.tensor.matmul(
                            ps_b[:],
                            lhsT=w_dec_sb[:, wj * A + aj * 128 : wj * A + (aj + 1) * 128],
                            rhs=decT_bf[:, wj * B_LOC : (wj + 1) * B_LOC],
                            start=(wj == 0), stop=False,
                        )
                    nc.tensor.matmul(
                        ps_b[:],
                        lhsT=bb_bf[0:1, aj * 128 : (aj + 1) * 128],
                        rhs=ones32[0:1, :],
                        start=False, stop=True,
                    )
                    nc.scalar.copy(biasT_sb[:, aj * B_LOC : (aj + 1) * B_LOC], ps_b[:])

            with (
                tc.tile_pool(name="nat_pool", bufs=6) as nat_pool,
                tc.tile_pool(name="natf8_pool", bufs=6) as natf8_pool,
                tc.tile_pool(name="encT_pool", bufs=3) as encT_pool,
                tc.tile_pool(name="hidT_pool", bufs=4) as hidT_pool,
                tc.tile_pool(name="w6_pool", bufs=6) as w6_pool,
                tc.tile_pool(name="mm_ps", bufs=2, space="PSUM") as mm_ps,
                tc.tile_pool(name="at_ps_pool", bufs=1, space="PSUM") as at_ps_pool,
                tc.tile_pool(name="acc_ps", bufs=1, space="PSUM") as acc_ps,
            ):
                out_ps = [
                    acc_ps.tile([128, 512], F32, name=f"out_ps{eg}") for eg in range(EG)
                ]
                z_ps = acc_ps.tile([128, 1], F32)

                nat = [None] * NCHUNK
                next6 = 0
                sizes = [1, 1, 2] + [4] * ((NCHUNK - 4) // 4)
                sizes += [NCHUNK - sum(sizes)] if sum(sizes) < NCHUNK else []
                assert sum(sizes) == NCHUNK
                starts = [sum(sizes[:i]) for i in range(len(sizes))]
                for g, (cstart, nch) in enumerate(zip(starts, sizes)):
                    gr = nch * 128
                    # encT8: [p, dj, (r two)] fp8 — pair (i=0,1) adjacent per row
                    encT = encT_pool.tile([128, DJ * 2 * 512], FP8, name="encT")
                    for pc in range(0, nch, 2):
                        c0 = cstart + pc
                        npair = min(2, nch - pc)
                        nat_t = nat_pool.tile([128, 2 * E], BF16, name="nat")
                        for i in range(npair):
                            nat[c0 + i] = nat_t[:, i * E : (i + 1) * E]
                        src = enc_x[c0 * 128 : (c0 + npair) * 128, :].rearrange(
                            "(i p) e -> p i e", p=128, i=npair
                        )
                        dst = nat_t.rearrange("p (i e) -> p i e", i=2)[:, 0:npair, :]
                        nc.gpsimd.dma_start(dst, src)
                        for i in range(npair):
                            rc = pc + i
                            # bf16 -> fp8 cast; alternate engines to split load
                            nf8 = natf8_pool.tile([128, E], FP8, name="natf8")
                            eng = nc.vector if (c0 + i) % 2 == 0 else nc.gpsimd
                            eng.tensor_copy(nf8[:], nat[c0 + i])
                            # u16-pair xbar transpose (one HWDGE ring only)
                            encT_3d = encT[:].bitcast(BF16).rearrange(
                                "p (j r) -> p j r", j=DJ
                            )
                            nc.sync.dma_start(
                                encT_3d[:, :, rc * 128 : rc * 128 + 128],
                                nf8[:].bitcast(BF16),
                                transpose=True,
                            )

                    hidT = hidT_pool.tile([128, AJ * 512], BF16, name="hidT")
                    encT_dj = encT[:].rearrange("p (dj rt) -> p dj rt", dj=DJ)
                    for aj in range(AJ):
                        ps_h = mm_ps.tile([128, 512], F32, name="ps_h")
                        for dj in range(DJ):
                            rhs = encT_dj[:, dj, 0 : 2 * gr].rearrange(
                                "p (r two) -> p two r", two=2
                            )
                            nc.tensor.matmul(
                                ps_h[:, 0:gr],
                                lhsT=w8_4d[:, dj, :, aj * 128 : (aj + 1) * 128],
                                rhs=rhs,
                                start=(dj == 0), stop=(dj == DJ - 1),
                                perf_mode=mybir.MatmulPerfMode.DoubleRow,
                            )
                        for b, s0, s1 in _batch_segments(128 * cstart, gr):
                            nc.scalar.activation(
                                hidT[:, aj * 512 + s0 : aj * 512 + s1],
                                ps_h[:, s0:s1],
                                AF.Relu,
                                bias=biasT_sb[:, aj * B_LOC + b : aj * B_LOC + b + 1],
                                scale=1.0 / WSCALE,
                            )

                    for rc in range(nch):
                        c = cstart + rc
                        at_ps = at_ps_pool.tile([128, 1], F32, name="at_ps")
                        for aj in range(AJ):
                            nc.tensor.matmul(
                                at_ps[:],
                                lhsT=hidT[:, aj * 512 + rc * 128 : aj * 512 + rc * 128 + 128],
                                rhs=wfin_sb[:, aj : aj + 1],
                                start=(aj == 0), stop=(aj == AJ - 1),
                            )
                        nc.scalar.activation(attT_sb[:, c : c + 1], at_ps[:], AF.Exp)

                    rows_done = 128 * (cstart + nch)
                    while next6 < NCHUNK:
                        last_b = (128 * next6 + 127) // P
                        if (last_b + 1) * P > rows_done:
                            break
                        c = next6
                        issue_mask(c)
                        w6 = w6_pool.tile([128, B_LOC], BF16, name="w6")
                        nc.vector.memset(w6[:], 0.0)
                        nc.vector.copy_predicated(
                            w6[:],
                            masks_sb[:, c * B_LOC : (c + 1) * B_LOC],
                            attT_sb[:, c : c + 1].broadcast_to([128, B_LOC]),
                        )
                        sj = (c % 4) * B_LOC
                        for eg in range(EG):
                            nc.tensor.matmul(
                                out_ps[eg][sj : sj + B_LOC, :],
                                lhsT=w6[:],
                                rhs=nat[c][:, eg * 512 : (eg + 1) * 512],
                                start=(c < 4), stop=(c >= NCHUNK - 4),
                                tile_position=(0, sj),
                                skip_group_check=True,
                            )
                        nc.tensor.matmul(
                            z_ps[sj : sj + B_LOC, :], lhsT=w6[:], rhs=onescol[:],
                            start=(c < 4), stop=(c >= NCHUNK - 4),
                            tile_position=(0, sj),
                            skip_group_check=True,
                        )
                        next6 += 1

                assert next6 == NCHUNK
                red_sb = consts.tile([128, EG * 512 + 1], F32, name="red_sb")
                for eg in range(EG):
                    nc.scalar.copy(red_sb[:, eg * 512 : (eg + 1) * 512], out_ps[eg][:])
                nc.vector.tensor_copy(red_sb[:, EG * 512 : EG * 512 + 1], z_ps[:])
                zf_ps = mm_ps.tile([B_LOC, 1], F32, name="ps_h")
                nc.tensor.matmul(
                    zf_ps[:], lhsT=id4[:], rhs=red_sb[:, EG * 512 : EG * 512 + 1],
                    start=True, stop=True,
                )
                nc.vector.reciprocal(recip_z[:], zf_ps[:])
                for eg in range(EG):
                    of_ps = mm_ps.tile([B_LOC, 512], F32, name="ps_h")
                    nc.tensor.matmul(
                        of_ps[:], lhsT=id4[:],
                        rhs=red_sb[:, eg * 512 : (eg + 1) * 512],
                        start=True, stop=True,
                    )
                    nc.scalar.activation(
                        out_sb[:, eg * 512 : (eg + 1) * 512],
                        of_ps[:],
                        AF.Copy,
                        scale=recip_z[:],
                    )
                if attT_x is not None:
                    attT_f = consts.tile([128, NCHUNK], F32, name="attT_f")
                    nc.vector.tensor_copy(attT_f[:], attT_sb[:])
                    nc.sync.dma_start(attT_x[:], attT_f[:])
                nc.sync.dma_start(out_x[:], out_sb[:])

    nc.compile()
    return nc


_NC = None


def _get_nc():
    global _NC
    if _NC is None:
        _NC = build()
    return _NC


def _make_w8(wenc):
    """W8[p, dj, i, m] = W_enc[256 dj + 2p + i, m] * WSCALE, fp8e4 as uint8."""
    fp8np = mybir.dt.np(FP8)
    w = (np.asarray(wenc, dtype=np.float32) * WSCALE).reshape(DJ, 128, 2, A)
    w8 = np.ascontiguousarray(w.transpose(1, 0, 2, 3)).astype(fp8np)
    return w8.reshape(128, DJ * 2 * A).view(np.uint8)


def _in_maps(inputs):
    enc = np.ascontiguousarray(np.asarray(inputs["encoder_out"], dtype=np.float32))
    dec = np.ascontiguousarray(np.asarray(inputs["decoder_out"], dtype=np.float32))
    w8 = _make_w8(inputs["W_enc"])
    benc = np.asarray(inputs["b_enc"], dtype=np.float32).reshape(1, A)
    wdec = np.ascontiguousarray(np.asarray(inputs["W_dec"], dtype=np.float32))
    bdec = np.asarray(inputs["b_dec"], dtype=np.float32).reshape(1, A)
    wfin = np.ascontiguousarray(np.asarray(inputs["W_fin"], dtype=np.float32))

    maps = []
    for i in range(N_CORES):
        maps.append(
            {
                "encoder_out": np.ascontiguousarray(
                    enc[i * B_LOC : (i + 1) * B_LOC].reshape(ROWS, E)
                ),
                "decoder_out": np.ascontiguousarray(dec[i * B_LOC : (i + 1) * B_LOC]),
                "W8": w8,
                "b_enc": benc,
                "W_dec": wdec,
                "b_dec": bdec,
                "W_fin": wfin,
            }
        )
    return maps


def run(inputs, trace=False):
    """Run the kernel; returns (out [256, 2048] f32, exec_time_ns or None)."""
    nc = _get_nc()
    res = run_bass_kernel_spmd(
        nc, _in_maps(inputs), core_ids=list(range(N_CORES)), trace=trace
    )
    out = np.concatenate([res.results[i]["out"] for i in range(N_CORES)], axis=0)
    return out.astype(np.float32), res.exec_time_ns


def kernel(**inputs):
    out, _ = run(inputs, trace=False)
    return out


# revision 4
# speedup vs baseline: 1.1742x; 1.1284x over previous
"""Trainium2 Bass kernel for the additive-attention module — fp8 variant.

Differences from the bf16 baseline (kernel.py):
  - The dominant matmul (att_enc = enc @ W_enc) runs in fp8e4 DoubleRow mode:
    256-deep contraction tiles, ~2x PE throughput.
  - W_enc is pre-scaled (x512), pair-interleaved, and cast to fp8 on the HOST
    (weights are small); shipped as uint8 and bitcast on chip.
    W8[p, dj, i, m] = W_enc[256 dj + 2p + i, m] * 512.
  - enc is cast bf16 -> fp8 on chip (DVE/Pool alternating), and the SBUF->SBUF
    xbar transposes move the fp8 data as u16 pairs: after transposing
    natf8.bitcast(bf16), fp8 element (p, 2r+i) of a 256-wide e-block equals
    enc[r, 256 dj + 2p + i] — exactly matching W8's pair layout.
    Transpose traffic halves vs bf16 (12.8 MB vs 25.7 MB per core).
  - bias+relu absorbs the 1/512 weight scale via the activation scale.
  - step4 (W_fin reduction), softmax, and step6 (weighted sum over bf16 nat)
    are unchanged from the baseline, keeping output precision at bf16 level.
"""

import sys

try:
    import concourse.bass as bass  # noqa: F401
except ImportError:
    sys.path.insert(0, "/opt/trn_rl_repo")

import numpy as np

import concourse.bass as bass
import concourse.mybir as mybir
import concourse.tile as tile
from concourse import bacc
from concourse.bass_utils import run_bass_kernel_spmd
from concourse.masks import make_identity

F32 = mybir.dt.float32
BF16 = mybir.dt.bfloat16
FP8 = mybir.dt.float8e4
U8 = mybir.dt.uint8
AF = mybir.ActivationFunctionType

N_CORES = 8
B = 256
B_LOC = B // N_CORES  # 32
P = 196
E = 2048
A = 512
W = 512
ROWS = B_LOC * P  # 6272
NCHUNK = (ROWS + 127) // 128  # 49
DJ = E // 256  # 8 double-row contraction tiles
AJ = A // 128  # 4
WJ = W // 128  # 4
EG = E // 512  # 4
WSCALE = 512.0


def _batch_segments(r0, nrows):
    """Batch segments of global row range [r0, r0+nrows): (batch, local_s0, local_s1)."""
    segs = []
    b0 = r0 // P
    b1 = (r0 + nrows - 1) // P
    for b in range(b0, b1 + 1):
        s0 = max(b * P - r0, 0)
        s1 = min((b + 1) * P - r0, nrows)
        if s1 > s0:
            segs.append((b, s0, s1))
    return segs


def build(debug_attT=False):
    nc = bacc.Bacc()

    enc_x = nc.dram_tensor("encoder_out", [ROWS, E], F32, kind="ExternalInput")
    attT_x = (
        nc.dram_tensor("attT_dbg", [128, NCHUNK], F32, kind="ExternalOutput")
        if debug_attT
        else None
    )
    dec_x = nc.dram_tensor("decoder_out", [B_LOC, W], F32, kind="ExternalInput")
    w8_x = nc.dram_tensor("W8", [128, DJ * 2 * A], U8, kind="ExternalInput")
    benc_x = nc.dram_tensor("b_enc", [1, A], F32, kind="ExternalInput")
    wdec_x = nc.dram_tensor("W_dec", [W, A], F32, kind="ExternalInput")
    bdec_x = nc.dram_tensor("b_dec", [1, A], F32, kind="ExternalInput")
    wfin_x = nc.dram_tensor("W_fin", [A], F32, kind="ExternalInput")
    out_x = nc.dram_tensor("out", [B_LOC, E], F32, kind="ExternalOutput")

    with tile.TileContext(nc) as tc:
        with tc.tile_pool(name="consts", bufs=1) as consts:
            identity = consts.tile([128, 128], F32)
            make_identity(nc, identity[:])
            identity_bf = consts.tile([128, 128], BF16)
            nc.vector.tensor_copy(identity_bf[:], identity[:])
            wfin_sb = consts.tile([128, AJ], BF16)
            nc.gpsimd.dma_start(wfin_sb[:], wfin_x.rearrange("(j p) -> p j", p=128))
            w8_sb = consts.tile([128, DJ * 2 * A], U8)
            nc.sync.dma_start(w8_sb[:], w8_x[:])
            # w8 as [p, dj, i, m] fp8
            w8_4d = w8_sb[:].bitcast(FP8).rearrange(
                "p (dj two m) -> p dj two m", dj=DJ, two=2
            )
            w_dec_sb = consts.tile([128, WJ * A], BF16)
            nc.gpsimd.dma_start(w_dec_sb[:], wdec_x.rearrange("(j p) a -> p j a", p=128))
            ones32 = consts.tile([1, 32], BF16)
            nc.vector.memset(ones32[:], 1.0)
            onescol = consts.tile([128, 1], F32)
            nc.vector.memset(onescol[:], 1.0)
            zacc = consts.tile([128, B_LOC], F32)
            nc.vector.memset(zacc[:], 0.0)

            dec_sb = consts.tile([B_LOC, W], F32)
            nc.sync.dma_start(dec_sb[:], dec_x[:])
            benc_sb = consts.tile([1, A], F32)
            nc.sync.dma_start(benc_sb[:], benc_x[:])
            bdec_sb = consts.tile([1, A], F32)
            nc.sync.dma_start(bdec_sb[:], bdec_x[:])
            bb_f = consts.tile([1, A], F32)
            nc.vector.tensor_add(bb_f[:], benc_sb[:], bdec_sb[:])
            bb_bf = consts.tile([1, A], BF16)
            nc.vector.tensor_copy(bb_bf[:], bb_f[:])

            decT_bf = consts.tile([128, WJ * B_LOC], BF16)
            biasT_sb = consts.tile([128, AJ * B_LOC], F32)
            attT_sb = consts.tile([128, NCHUNK], BF16)
            out_sb = consts.tile([B_LOC, E], F32)
            recip_z = consts.tile([B_LOC, 1], F32)

            # per-chunk batch-membership masks (see kernel.py for the scheme)
            id4 = consts.tile([128, B_LOC], F32)
            nc.gpsimd.memset(id4[:], 0.0)
            for k in range(4):
                nc.gpsimd.affine_select(
                    id4[:], id4[:], pattern=[[-1, B_LOC]],
                    compare_op=mybir.AluOpType.not_equal, fill=1.0,
                    base=-B_LOC * k, channel_multiplier=1,
                )
            ones_pb = consts.tile([128, B_LOC], mybir.dt.int8)
            nc.vector.memset(ones_pb[:], 1)
            masks_sb = consts.tile([128, NCHUNK * B_LOC], mybir.dt.int8)

            def issue_mask(c):
                m = masks_sb[:, c * B_LOC : (c + 1) * B_LOC]
                nc.gpsimd.affine_select(
                    m, ones_pb[:], pattern=[[-P, B_LOC]],
                    compare_op=mybir.AluOpType.is_ge, fill=0.0,
                    base=128 * c, channel_multiplier=1,
                )
                nc.gpsimd.affine_select(
                    m, m, pattern=[[P, B_LOC]],
                    compare_op=mybir.AluOpType.is_ge, fill=0.0,
                    base=(P - 1) - 128 * c, channel_multiplier=-1,
                )

            # prologue: decT, then biasT = (dec @ W_dec + b_dec + b_enc).T  [a, b]
            with tc.tile_pool(name="pro_ps", bufs=2, space="PSUM") as pro_ps:
                for j in range(WJ):
                    ps_dt = pro_ps.tile([128, B_LOC], F32, name="ps_dt")
                    nc.tensor.transpose(
                        ps_dt[:], dec_sb[0:B_LOC, j * 128 : (j + 1) * 128],
                        identity[0:B_LOC, 0:B_LOC],
                    )
                    nc.vector.tensor_copy(decT_bf[:, j * B_LOC : (j + 1) * B_LOC], ps_dt[:])
                for aj in range(AJ):
                    ps_b = pro_ps.tile([128, B_LOC], F32, name="ps_b")
                    for wj in range(WJ):
                        nc.tensor.matmul(
                            ps_b[:],
                            lhsT=w_dec_sb[:, wj * A + aj * 128 : wj * A + (aj + 1) * 128],
                            rhs=decT_bf[:, wj * B_LOC : (wj + 1) * B_LOC],
                            start=(wj == 0), stop=False,
                        )
                    nc.tensor.matmul(
                        ps_b[:],
                        lhsT=bb_bf[0:1, aj * 128 : (aj + 1) * 128],
                        rhs=ones32[0:1, :],
                        start=False, stop=True,
                    )
                    nc.scalar.copy(biasT_sb[:, aj * B_LOC : (aj + 1) * B_LOC], ps_b[:])

            with (
                tc.tile_pool(name="nat_pool", bufs=6) as nat_pool,
                tc.tile_pool(name="natf8_pool", bufs=6) as natf8_pool,
                tc.tile_pool(name="encT_pool", bufs=3) as encT_pool,
                tc.tile_pool(name="hidT_pool", bufs=4) as hidT_pool,
                tc.tile_pool(name="w6_pool", bufs=6) as w6_pool,
                tc.tile_pool(name="mm_ps", bufs=2, space="PSUM") as mm_ps,
                tc.tile_pool(name="aux_ps", bufs=2, space="PSUM") as aux_ps,
                tc.tile_pool(name="acc_ps", bufs=1, space="PSUM") as acc_ps,
            ):
                out_ps = [
                    acc_ps.tile([128, 512], F32, name=f"out_ps{eg}") for eg in range(EG)
                ]

                nat = [None] * NCHUNK
                next6 = 0
                sizes = [1, 1, 2] + [4] * ((NCHUNK - 4) // 4)
                sizes += [NCHUNK - sum(sizes)] if sum(sizes) < NCHUNK else []
                assert sum(sizes) == NCHUNK
                starts = [sum(sizes[:i]) for i in range(len(sizes))]
                for g, (cstart, nch) in enumerate(zip(starts, sizes)):
                    gr = nch * 128
                    # encT8: [p, dj, (r two)] fp8 — pair (i=0,1) adjacent per row
                    encT = encT_pool.tile([128, DJ * 2 * 512], FP8, name="encT")
                    for pc in range(0, nch, 2):
                        c0 = cstart + pc
                        npair = min(2, nch - pc)
                        nat_t = nat_pool.tile([128, 2 * E], BF16, name="nat")
                        for i in range(npair):
                            nat[c0 + i] = nat_t[:, i * E : (i + 1) * E]
                        src = enc_x[c0 * 128 : (c0 + npair) * 128, :].rearrange(
                            "(i p) e -> p i e", p=128, i=npair
                        )
                        dst = nat_t.rearrange("p (i e) -> p i e", i=2)[:, 0:npair, :]
                        nc.gpsimd.dma_start(dst, src)
                        for i in range(npair):
                            rc = pc + i
                            # bf16 -> fp8 cast; alternate engines to split load
                            nf8 = natf8_pool.tile([128, E], FP8, name="natf8")
                            eng = nc.vector if (c0 + i) % 2 == 0 else nc.gpsimd
                            eng.tensor_copy(nf8[:], nat[c0 + i])
                            # u16-pair transpose on the PE (transpose-mode is
                            # a raw-element permutation, fp8-pair safe); keeps
                            # the shared SDMA engines free for the SWDGE enc
                            # stream, which xbar transposes do NOT overlap with
                            encT_3d = encT[:].bitcast(BF16).rearrange(
                                "p (j r) -> p j r", j=DJ
                            )
                            nf8_bf = nf8[:].bitcast(BF16)
                            for h in range(2):
                                trp = aux_ps.tile([128, 4 * 128], BF16, name="trp")
                                for k in range(4):
                                    dj = 4 * h + k
                                    nc.tensor.transpose(
                                        trp[:, k * 128 : (k + 1) * 128],
                                        nf8_bf[:, dj * 128 : (dj + 1) * 128],
                                        identity_bf[:],
                                    )
                                dst = encT_3d[:, 4 * h : 4 * h + 4,
                                              rc * 128 : rc * 128 + 128]
                                srcv = trp[:].rearrange("p (j r) -> p j r", j=4)
                                # GPSIMD cannot read PSUM: drain on DVE/ACT
                                if (c0 + i) % 2 == 1:
                                    nc.vector.tensor_copy(dst, srcv)
                                else:
                                    nc.scalar.copy(dst, srcv)

                    hidT = hidT_pool.tile([128, AJ * 512], BF16, name="hidT")
                    encT_dj = encT[:].rearrange("p (dj rt) -> p dj rt", dj=DJ)
                    for aj in range(AJ):
                        ps_h = mm_ps.tile([128, 512], F32, name="ps_h")
                        for dj in range(DJ):
                            rhs = encT_dj[:, dj, 0 : 2 * gr].rearrange(
                                "p (r two) -> p two r", two=2
                            )
                            nc.tensor.matmul(
                                ps_h[:, 0:gr],
                                lhsT=w8_4d[:, dj, :, aj * 128 : (aj + 1) * 128],
                                rhs=rhs,
                                start=(dj == 0), stop=(dj == DJ - 1),
                                perf_mode=mybir.MatmulPerfMode.DoubleRow,
                            )
                        for b, s0, s1 in _batch_segments(128 * cstart, gr):
                            nc.scalar.activation(
                                hidT[:, aj * 512 + s0 : aj * 512 + s1],
                                ps_h[:, s0:s1],
                                AF.Relu,
                                bias=biasT_sb[:, aj * B_LOC + b : aj * B_LOC + b + 1],
                                scale=1.0 / WSCALE,
                            )

                    for rc in range(nch):
                        c = cstart + rc
                        at_ps = mm_ps.tile([128, 1], F32, name="ps_h")
                        for aj in range(AJ):
                            nc.tensor.matmul(
                                at_ps[:],
                                lhsT=hidT[:, aj * 512 + rc * 128 : aj * 512 + rc * 128 + 128],
                                rhs=wfin_sb[:, aj : aj + 1],
                                start=(aj == 0), stop=(aj == AJ - 1),
                            )
                        nc.scalar.activation(attT_sb[:, c : c + 1], at_ps[:], AF.Exp)

                    rows_done = 128 * (cstart + nch)
                    while next6 < NCHUNK:
                        last_b = (128 * next6 + 127) // P
                        if (last_b + 1) * P > rows_done:
                            break
                        c = next6
                        issue_mask(c)
                        w6 = w6_pool.tile([128, B_LOC], BF16, name="w6")
                        nc.vector.memset(w6[:], 0.0)
                        nc.vector.copy_predicated(
                            w6[:],
                            masks_sb[:, c * B_LOC : (c + 1) * B_LOC],
                            attT_sb[:, c : c + 1].broadcast_to([128, B_LOC]),
                        )
                        nc.vector.tensor_add(zacc[:], zacc[:], w6[:])
                        sj = (c % 4) * B_LOC
                        for eg in range(EG):
                            nc.tensor.matmul(
                                out_ps[eg][sj : sj + B_LOC, :],
                                lhsT=w6[:],
                                rhs=nat[c][:, eg * 512 : (eg + 1) * 512],
                                start=(c < 4), stop=(c >= NCHUNK - 4),
                                tile_position=(0, sj),
                                skip_group_check=True,
                            )
                        next6 += 1

                assert next6 == NCHUNK
                red_sb = consts.tile([128, EG * 512], F32, name="red_sb")
                for eg in range(EG):
                    nc.scalar.copy(red_sb[:, eg * 512 : (eg + 1) * 512], out_ps[eg][:])
                zf_ps = mm_ps.tile([B_LOC, 1], F32, name="ps_h")
                nc.tensor.matmul(
                    zf_ps[:], lhsT=zacc[:], rhs=onescol[:],
                    start=True, stop=True,
                )
                nc.vector.reciprocal(recip_z[:], zf_ps[:])
                for eg in range(EG):
                    of_ps = mm_ps.tile([B_LOC, 512], F32, name="ps_h")
                    nc.tensor.matmul(
                        of_ps[:], lhsT=id4[:],
                        rhs=red_sb[:, eg * 512 : (eg + 1) * 512],
                        start=True, stop=True,
                    )
                    nc.scalar.activation(
                        out_sb[:, eg * 512 : (eg + 1) * 512],
                        of_ps[:],
                        AF.Copy,
                        scale=recip_z[:],
                    )
                if attT_x is not None:
                    attT_f = consts.tile([128, NCHUNK], F32, name="attT_f")
                    nc.vector.tensor_copy(attT_f[:], attT_sb[:])
                    nc.sync.dma_start(attT_x[:], attT_f[:])
                nc.sync.dma_start(out_x[:], out_sb[:])

    nc.compile()
    return nc


_NC = None


def _get_nc():
    global _NC
    if _NC is None:
        _NC = build()
    return _NC


def _make_w8(wenc):
    """W8[p, dj, i, m] = W_enc[256 dj + 2p + i, m] * WSCALE, fp8e4 as uint8."""
    fp8np = mybir.dt.np(FP8)
    w = (np.asarray(wenc, dtype=np.float32) * WSCALE).reshape(DJ, 128, 2, A)
    w8 = np.ascontiguousarray(w.transpose(1, 0, 2, 3)).astype(fp8np)
    return w8.reshape(128, DJ * 2 * A).view(np.uint8)


def _in_maps(inputs):
    enc = np.ascontiguousarray(np.asarray(inputs["encoder_out"], dtype=np.float32))
    dec = np.ascontiguousarray(np.asarray(inputs["decoder_out"], dtype=np.float32))
    w8 = _make_w8(inputs["W_enc"])
    benc = np.asarray(inputs["b_enc"], dtype=np.float32).reshape(1, A)
    wdec = np.ascontiguousarray(np.asarray(inputs["W_dec"], dtype=np.float32))
    bdec = np.asarray(inputs["b_dec"], dtype=np.float32).reshape(1, A)
    wfin = np.ascontiguousarray(np.asarray(inputs["W_fin"], dtype=np.float32))

    maps = []
    for i in range(N_CORES):
        maps.append(
            {
                "encoder_out": np.ascontiguousarray(
                    enc[i * B_LOC : (i + 1) * B_LOC].reshape(ROWS, E)
                ),
                "decoder_out": np.ascontiguousarray(dec[i * B_LOC : (i + 1) * B_LOC]),
                "W8": w8,
                "b_enc": benc,
                "W_dec": wdec,
                "b_dec": bdec,
                "W_fin": wfin,
            }
        )
    return maps


def run(inputs, trace=False):
    """Run the kernel; returns (out [256, 2048] f32, exec_time_ns or None)."""
    nc = _get_nc()
    res = run_bass_kernel_spmd(
        nc, _in_maps(inputs), core_ids=list(range(N_CORES)), trace=trace
    )
    out = np.concatenate([res.results[i]["out"] for i in range(N_CORES)], axis=0)
    return out.astype(np.float32), res.exec_time_ns


def kernel(**inputs):
    out, _ = run(inputs, trace=False)
    return out
